# revision 1
# baseline (speedup 1.0000x reference)
"""Trainium2 Bass kernel for nn_MultiHeadCrossAttention (BS=4, S=512, DM=512, H=8).

Sharding: one attention head per NeuronCore (8 heads / 8 cores). Each core
receives the full (transposed) q/k/v plus its head's weight slices, computes
its head end-to-end including the rank-64 slice of the output projection, and
the host sums the 8 partial outputs.

Math restructuring (validated against the reference numerically):
  E^T[kb][j,i] = exp(khT[kb]^T qhT[b]) computed per q-batch b in transposed
  layout; fenmu handled as W = 1/sum_kb E^T (the sqrt(DK)=8 factor is folded
  into Wv/bv on the host); RT = E^T[b] * W; score[i,(c,d)] = RT^T @ vh.
  softmax+LN over d uses: mean(sm) = 1/DK exactly; Sum(sm^2) = Q/Z^2 with
  Z = sum exp(score), Q = sum exp(2*score); std = exp(0.5*ln(63*var) +
  0.5*ln(1/63)); LN sum over c collapses to  sum_c e_c * w1_c  + w0, applied
  via per-partition tensor_scalar (gpsimd) and PE matmul accumulations.
"""

import numpy as np

BS, S, DM, H, DK = 4, 512, 512, 8, 64
EPS = 1e-6
NCORES = 8

F32 = None  # set lazily (mybir import)


def build_program(nc, tile, mybir):
    f32 = mybir.dt.float32
    bf16 = mybir.dt.bfloat16
    i32 = mybir.dt.int32
    AF = mybir.ActivationFunctionType
    OP = mybir.AluOpType
    AX = mybir.AxisListType

    # ---- DRAM I/O (host pre-layouts everything for contiguous DMA) ----
    qT_d = nc.dram_tensor("qT", [BS, 128, 4, S], bf16, kind="ExternalInput")
    kT_d = nc.dram_tensor("kT", [BS, 128, 4, S], bf16, kind="ExternalInput")
    vT_d = nc.dram_tensor("vT", [BS, 128, 4, S], bf16, kind="ExternalInput")
    Wq_d = nc.dram_tensor("Wq", [128, 4, DK], bf16, kind="ExternalInput")
    Wk_d = nc.dram_tensor("Wk", [128, 4, DK], bf16, kind="ExternalInput")
    Wv_d = nc.dram_tensor("Wv", [128, 4, DK], bf16, kind="ExternalInput")
    bqc_d = nc.dram_tensor("bqc", [DK, 1], f32, kind="ExternalInput")
    bkc_d = nc.dram_tensor("bkc", [DK, 1], f32, kind="ExternalInput")
    bv_d = nc.dram_tensor("bv", [1, DK], bf16, kind="ExternalInput")
    Wo_d = nc.dram_tensor("Wo", [DK, DM], bf16, kind="ExternalInput")
    Wo4_d = nc.dram_tensor("Wo4", [DK, DM], bf16, kind="ExternalInput")
    bo2_d = nc.dram_tensor("bo2", [128, 4], f32, kind="ExternalInput")
    al_d = nc.dram_tensor("alpha", [DK, 1], f32, kind="ExternalInput")
    b4_d = nc.dram_tensor("beta4", [DK, 1], f32, kind="ExternalInput")
    id_d = nc.dram_tensor("ident", [128, 128], bf16, kind="ExternalInput")
    idf_d = nc.dram_tensor("identf", [128, 128], f32, kind="ExternalInput")
    outT_d = nc.dram_tensor("outT", [BS, DM, S], bf16, kind="ExternalOutput")

    class _scope:
        def __init__(self, name):
            self.name = name
        def __enter__(self):
            self.sid, _ = nc.enter_named_scope(self.name, False)
        def __exit__(self, *a):
            nc.leave_named_scope(self.name, self.sid, False)

    with tile.TileContext(nc) as tc:
        with (
            tc.tile_pool(name="persist", bufs=1) as pp,
            tc.tile_pool(name="consts", bufs=1) as cp,
            tc.tile_pool(name="inp", bufs=3) as inp,
            tc.tile_pool(name="work", bufs=6) as wp,
            tc.tile_pool(name="vt", bufs=1) as vtp,
            tc.tile_pool(name="bwork", bufs=3) as bwp,
            tc.tile_pool(name="psum", bufs=1, space="PSUM") as psp,
        ):
            # ---- persistent SBUF ----
            qhT = pp.tile([DK, BS, S], bf16, tag="qhT")
            khT = pp.tile([DK, BS, S], bf16, tag="khT")
            vh_all = pp.tile([128, 4, BS, DK], bf16, tag="vh")   # [j128, jc, c, d]
            rt_all = pp.tile([128, BS, 4, S], bf16, tag="rt")    # [j128, b, jc, i]
            e_all = pp.tile([128, BS, 4, 2 * BS * DK], f32, tag="e")  # [i,b,ic,(u,c,d)]
            heads = pp.tile([DK, BS, S], bf16, tag="heads")       # [d, b, i]
            Z_all = pp.tile([128, 64], f32, tag="Z")   # cols = b*16 + ic*4 + c
            Q_all = pp.tile([128, 64], f32, tag="Q")
            w1_all = pp.tile([128, 64], f32, tag="w1")
            w0_all = pp.tile([128, 16], f32, tag="w0")  # cols = b*4 + ic
            w0T0 = pp.tile([8, 128], f32, tag="w0T0")
            w0T1 = pp.tile([8, 128], f32, tag="w0T1")
            w0f = pp.tile([1, 16 * 128], f32, tag="w0f")

            Wq_s = cp.tile([128, 4, DK], bf16, tag="Wq")
            Wk_s = cp.tile([128, 4, DK], bf16, tag="Wk")
            Wv_s = cp.tile([128, 4, DK], bf16, tag="Wv")
            Wo_s = cp.tile([DK, DM], bf16, tag="Wo")
            Wo4_s = cp.tile([DK, DM], bf16, tag="Wo4")
            bo2_s = cp.tile([128, 4], f32, tag="bo2")
            bqc_s = cp.tile([DK, 1], f32, tag="bqc")
            bkc_s = cp.tile([DK, 1], f32, tag="bkc")
            bv_s = cp.tile([1, DK], bf16, tag="bv")
            al_s = cp.tile([DK, 1], f32, tag="al")
            b4_s = cp.tile([DK, 1], f32, tag="b4")
            id_s = cp.tile([128, 128], bf16, tag="id")
            idf_s = cp.tile([128, 128], f32, tag="idf")
            ones = cp.tile([1, S], bf16, tag="ones")
            ones_f = cp.tile([1, S], f32, tag="ones_f")
            bvb = cp.tile([128, DK], f32, tag="bvb")

            # ---- input k tiles + weights first: they gate the first projs ----
            ktiles = []
            for b in range(BS):
                kt_early = inp.tile([128, 4, S], bf16, tag="kt", name=f"kte{b}")
                nc.sync.dma_start(kt_early[:], kT_d[b])
                ktiles.append(kt_early)
            nc.sync.dma_start(Wk_s[:], Wk_d[:])
            nc.sync.dma_start(Wq_s[:], Wq_d[:])
            nc.sync.dma_start(Wv_s[:], Wv_d[:])
            nc.sync.dma_start(bqc_s[:], bqc_d[:])
            nc.sync.dma_start(bkc_s[:], bkc_d[:])
            nc.sync.dma_start(bv_s[:], bv_d[:])
            nc.sync.dma_start(id_s[:], id_d[:])
            nc.sync.dma_start(idf_s[:], idf_d[:])
            nc.sync.dma_start(Wo_s[:], Wo_d[:])
            nc.sync.dma_start(Wo4_s[:], Wo4_d[:])
            nc.sync.dma_start(bo2_s[:], bo2_d[:])
            nc.sync.dma_start(al_s[:], al_d[:])
            nc.sync.dma_start(b4_s[:], b4_d[:])
            nc.vector.memset(ones[:], 1.0)
            nc.vector.memset(ones_f[:], 1.0)
            nc.scalar.activation(ones_f[0:1, 0:8], ones_f[0:1, 0:8], AF.Exp)
            nc.vector.memset(ones_f[0:1, 0:8], 1.0)

            # Unified PSUM tags:  pe0/pe1: [128,2,512] = 4 banks | su: 2x = 2
            #                     sc: 2x = 2   (total 8 banks)
            def psum_su():
                return psp.tile([128, S], f32, tag="su", bufs=2, name="psu")
            def psum_sc():
                return psp.tile([128, S], f32, tag="sc", bufs=2, name="psc")
            vtiles = []

            # ---- P1: k, q projections ----
            def emit_proj(W_s, b_c, dsrc, tag, dst, b):
                src = inp.tile([128, 4, S], bf16, tag=tag)
                nc.sync.dma_start(src[:], dsrc[b])
                ps = psp.tile([DK, S], f32, tag="su", bufs=2, name="pproj")
                for mc in range(4):
                    nc.tensor.matmul(ps[:], W_s[:, mc, :], src[:, mc, :],
                                     start=(mc == 0), stop=(mc == 3))
                nc.scalar.activation(dst[:, b, :], ps[:], AF.Identity, bias=b_c[:])

            with _scope("P1"):
                for b in range(BS):
                    ps = psp.tile([DK, S], f32, tag="su", bufs=2, name="pprojk")
                    for mc in range(4):
                        nc.tensor.matmul(ps[:], Wk_s[:, mc, :],
                                         ktiles[b][:, mc, :],
                                         start=(mc == 0), stop=(mc == 3))
                    nc.scalar.activation(khT[:, b, :], ps[:], AF.Identity,
                                         bias=bkc_s[:])
                for b in range(BS):
                    emit_proj(Wq_s, bqc_s, qT_d, "qt", qhT, b)
                for b in range(BS):
                    vt = vtp.tile([128, 4, S], bf16, tag=f"vt{b}")
                    nc.sync.dma_start(vt[:], vT_d[b])
                    vtiles.append(vt)

            # ---- P2/P3 machinery ----
            def p2_tail(b, jc, ex):
                su = psum_su()
                for kb in range(4):
                    nc.tensor.matmul(su[:], id_s[:], ex[:, kb, :],
                                     start=(kb == 0), stop=(kb == 3))
                wrec = wp.tile([128, S], f32, tag="wrec")
                nc.vector.reciprocal_approx_fast(wrec[:], su[:])
                eng = nc.gpsimd if jc % 2 == 0 else nc.vector
                eng.tensor_tensor(
                    rt_all[:, b, jc, :], ex[:, b, :], wrec[:], op=OP.mult)

            def emit_p2(b):
                pend = None
                for jc in range(4):
                    ex = wp.tile([128, 4, S], bf16, tag="ex")
                    for half in range(2):
                        ph = psp.tile([128, 2, S], f32, tag=f"pe{half}",
                                      name="ppe")
                        for k2 in range(2):
                            kb = half * 2 + k2
                            nc.tensor.matmul(
                                ph[:, k2, :],
                                khT[:, kb, jc * 128:(jc + 1) * 128],
                                qhT[:, b, :],
                                start=True, stop=True,
                            )
                        nc.scalar.activation(
                            ex[:, half * 2:(half + 1) * 2, :], ph[:], AF.Exp)
                    if pend is not None:
                        p2_tail(*pend)
                    pend = (b, jc, ex)
                p2_tail(*pend)

            def emit_p3(b):
                for ic in range(4):
                    pc = psum_sc()
                    for jc in range(4):
                        nc.tensor.matmul(
                            pc[:, 0:BS * DK],
                            rt_all[:, b, jc, ic * 128:(ic + 1) * 128],
                            vh_all[:, jc].rearrange("p c d -> p (c d)"),
                            start=(jc == 0), stop=(jc == 3),
                        )
                    nc.scalar.activation(e_all[:, b, ic, 0:BS * DK],
                                         pc[:, 0:BS * DK], AF.Exp)
                    nc.scalar.activation(e_all[:, b, ic, BS * DK:2 * BS * DK],
                                         pc[:, 0:BS * DK], AF.Exp, scale=2.0)
                    col = b * 16 + ic * 4
                    zq = wp.tile([128, 2, 4], f32, tag="zq")
                    nc.vector.tensor_reduce(
                        zq[:],
                        e_all[:, b, ic, :].rearrange("p (u g d) -> p u g d", u=2,
                                                     d=DK),
                        axis=AX.X, op=OP.add,
                    )
                    nc.vector.tensor_copy(Z_all[:, col:col + 4], zq[:, 0, :])
                    nc.vector.tensor_copy(Q_all[:, col:col + 4], zq[:, 1, :])

            def emit_vh():
                # bv broadcast
                pb = psum_sc()
                nc.tensor.matmul(pb[:, 0:DK], ones[:, 0:128], bv_s[:],
                                 start=True, stop=True)
                nc.vector.tensor_copy(bvb[:], pb[:, 0:DK])
                for c in range(BS):
                    vt = vtiles[c]
                    for jc in range(4):
                        pv = psum_sc()
                        for mc in range(4):
                            nc.tensor.matmul(
                                pv[:, 0:DK], vt[:, mc, jc * 128:(jc + 1) * 128],
                                Wv_s[:, mc, :],
                                start=(mc == 0), stop=(mc == 3),
                            )
                        nc.vector.tensor_tensor(
                            vh_all[:, jc, c, :], pv[:, 0:DK], bvb[:], op=OP.add)


            # ---- P4: stats per b-pair (overlaps P23 tail) ----
            stp_cm = tc.tile_pool(name="stats", bufs=2)
            stp = stp_cm.__enter__()

            def emit_stats(h):
                c0, c1 = h * 32, (h + 1) * 32
                Zs, Qs = Z_all[:, c0:c1], Q_all[:, c0:c1]
                t = stp.tile([128, 32], f32, tag="t", name="t")
                nc.vector.tensor_tensor(t[:], Zs, Zs, op=OP.mult)
                s = stp.tile([128, 32], f32, tag="s", name="s")
                nc.vector.scalar_tensor_tensor(
                    s[:], t[:], -1.0 / DK, Qs, op0=OP.mult, op1=OP.add)
                rinv = stp.tile([128, 32], f32, tag="rinv", name="rinv")
                nc.vector.reciprocal(rinv[:], t[:])
                v63 = stp.tile([128, 32], f32, tag="v63", name="v63")
                nc.vector.tensor_tensor(v63[:], s[:], rinv[:], op=OP.mult)
                r_ = stp.tile([128, 32], f32, tag="r_", name="r_")
                nc.vector.tensor_scalar(r_[:].bitcast(i32), v63[:].bitcast(i32),
                                        1, None, op0=OP.logical_shift_right)
                nc.vector.tensor_scalar(r_[:].bitcast(i32), r_[:].bitcast(i32),
                                        -1, 0x5F3759DF, op0=OP.mult, op1=OP.add)
                nt = stp.tile([128, 32], f32, tag="nt", name="nt")
                for _ in range(2):
                    nc.vector.tensor_tensor(nt[:], v63[:], r_[:], op=OP.mult)
                    nc.vector.tensor_tensor(nt[:], nt[:], r_[:], op=OP.mult)
                    nc.vector.tensor_scalar(nt[:], nt[:], -0.5, 1.5,
                                            op0=OP.mult, op1=OP.add)
                    nc.vector.tensor_tensor(r_[:], r_[:], nt[:], op=OP.mult)
                R_ = stp.tile([128, 32], f32, tag="R_", name="R_")
                nc.vector.tensor_scalar(R_[:], r_[:], float(np.sqrt(DK - 1.0)),
                                        None, op0=OP.mult)
                u_ = stp.tile([128, 32], f32, tag="u_", name="u_")
                nc.vector.tensor_scalar(u_[:], R_[:], -EPS, 1.0,
                                        op0=OP.mult, op1=OP.add)
                g = stp.tile([128, 32], f32, tag="g", name="g")
                nc.vector.tensor_tensor(g[:], R_[:], u_[:], op=OP.mult)
                zr = stp.tile([128, 32], f32, tag="zr", name="zr")
                nc.vector.reciprocal(zr[:], Zs)
                nc.vector.tensor_tensor(w1_all[:, c0:c1], g[:], zr[:], op=OP.mult)
                gs = stp.tile([128, 8], f32, tag="gs", name="gs")
                nc.vector.tensor_reduce(
                    gs[:], g[:].rearrange("p (s c) -> p s c", c=4), axis=AX.X,
                    op=OP.add)
                nc.vector.tensor_scalar(w0_all[:, h * 8:(h + 1) * 8], gs[:],
                                        -1.0 / DK, None, op0=OP.mult)
                pw = psum_sc()
                nc.tensor.matmul(pw[:8, 0:128], w0_all[:, h * 8:(h + 1) * 8],
                                 idf_s[:], is_transpose=True, start=True,
                                 stop=True)
                w0Th = w0T0 if h == 0 else w0T1
                nc.vector.tensor_copy(w0Th[:, :], pw[:8, 0:128])
                nc.sync.dma_start(
                    w0f[0:1, h * 1024:(h + 1) * 1024]
                    .rearrange("o (s f) -> o s f", s=8),
                    w0Th[:, :])

            # ---- P5 + P6 per-b (P5 DVE ops batched per b) ----
            def emit_p5(b):
                bsc = bwp.tile([128, 4, 4, DK], f32, tag="bsc")  # [i, ic, c, d]
                w1b = (w1_all[:, b * 16:(b + 1) * 16]
                       .rearrange("p (i c) -> p i c", c=4)
                       .unsqueeze(-1).broadcast_to((128, 4, 4, DK)))
                nc.vector.tensor_tensor(
                    bsc[:],
                    e_all[:, b, :, 0:BS * DK].rearrange("p i (c d) -> p i c d",
                                                        d=DK),
                    w1b, op=OP.mult,
                )
                ball = bwp.tile([128, 4, DK], f32, tag="ball")  # [i, ic, d]
                nc.vector.tensor_reduce(
                    ball[:], bsc[:].rearrange("p i c d -> p i d c"),
                    axis=AX.X, op=OP.add,
                )
                # all 4 transposes + w0 rank-1 adds into ONE psum bank
                pbig = psp.tile([DK, S], f32, tag="sc", bufs=2, name="pbig")
                for ic in range(4):
                    nc.tensor.matmul(pbig[:, ic * 128:(ic + 1) * 128],
                                     ball[:, ic, :], idf_s[:],
                                     is_transpose=True, start=True, stop=False)
                    slot = b * 4 + ic
                    nc.tensor.matmul(
                        pbig[:, ic * 128:(ic + 1) * 128], ones_f[:, 0:DK],
                        w0f[0:1, slot * 128:(slot + 1) * 128],
                        start=False, stop=True,
                    )
                nc.vector.tensor_scalar(
                    heads[:, b, :], pbig[:],
                    al_s[:], b4_s[:], op0=OP.mult, op1=OP.add,
                )

            def emit_p6(b):
                for nch in range(4):
                    po = psum_su()
                    nc.tensor.matmul(
                        po[:], Wo_s[:, nch * 128:(nch + 1) * 128],
                        heads[:, b, :], start=True, stop=False,
                    )
                    nc.tensor.matmul(
                        po[:], Wo4_s[:, nch * 128:(nch + 1) * 128],
                        qhT[:, b, :], start=False, stop=True,
                    )
                    ot = bwp.tile([128, S], bf16, tag="ot")
                    nc.scalar.activation(ot[:], po[:], AF.Identity,
                                         bias=bo2_s[:, nch:nch + 1])
                    nc.sync.dma_start(outT_d[b, nch * 128:(nch + 1) * 128, :], ot[:])

            with _scope("P2356"):
                emit_p2(0)
                emit_vh()
                emit_p2(1)
                emit_p3(0)
                emit_p2(2)
                emit_p3(1)
                emit_p2(3)
                emit_p3(2)
                emit_p3(3)
                emit_stats(0)
                emit_stats(1)
                emit_p5(0)
                emit_p5(1)
                emit_p6(0)
                emit_p5(2)
                emit_p6(1)
                emit_p5(3)
                emit_p6(2)
                emit_p6(3)
            stp_cm.__exit__(None, None, None)

    nc._dbg_names = {
        "qhT": qhT.name, "khT": khT.name, "vh_all": vh_all.name,
        "rt_all": rt_all.name, "e_all": e_all.name, "heads": heads.name,
        "Z_all": Z_all.name, "Q_all": Q_all.name, "w1_all": w1_all.name,
        "w0_all": w0_all.name, "w0f": w0f.name,
    }
    return nc


def _build():
    import concourse.bass as bass  # noqa
    import concourse.tile as tile
    from concourse import bacc, mybir

    nc = bacc.Bacc("TRN2", target_bir_lowering=False, debug=False,
                   num_devices=NCORES)
    build_program(nc, tile, mybir)
    nc.compile()
    return nc


_cached_nc = None


def make_in_maps(q, k, v, Wq, bq, Wk, bk, Wv, bv, Wo, bo, alpha, beta):
    import ml_dtypes
    bft = ml_dtypes.bfloat16

    def prelay(x):
        # [S, DM] per batch -> transposed [DM, S] -> [128, 4, S] partition layout
        xT = np.swapaxes(np.asarray(x, np.float32), 1, 2)  # [B, DM, S]
        return np.ascontiguousarray(
            xT.reshape(BS, 4, 128, S).transpose(0, 2, 1, 3)).astype(bft)

    def wlay(W):  # [DM, DK] -> [128, 4, DK]
        return np.ascontiguousarray(
            np.asarray(W, np.float32).reshape(4, 128, DK).transpose(1, 0, 2)
        ).astype(bft)

    qT, kT, vT = prelay(q), prelay(k), prelay(v)
    Wq, Wk, Wv, Wo = (np.asarray(x, np.float32) for x in (Wq, Wk, Wv, Wo))
    bq, bk, bv, bo = (np.asarray(x, np.float32) for x in (bq, bk, bv, bo))
    alpha, beta = np.asarray(alpha, np.float32), np.asarray(beta, np.float32)
    ident = np.eye(128, dtype=ml_dtypes.bfloat16)
    identf = np.eye(128, dtype=np.float32)
    scale = np.float32(1.0 / np.sqrt(np.float32(DK)))  # fenmu sqrt(DK), into Wv
    in_maps = []
    for h in range(NCORES):
        sl = slice(h * DK, (h + 1) * DK)
        in_maps.append({
            "qT": qT, "kT": kT, "vT": vT,
            "Wq": wlay(Wq[:, sl]),
            "Wk": wlay(Wk[:, sl]),
            "Wv": wlay(Wv[:, sl] * scale),
            "bqc": np.ascontiguousarray(bq[sl])[:, None].astype(np.float32),
            "bkc": np.ascontiguousarray(bk[sl])[:, None].astype(np.float32),
            "bv": np.ascontiguousarray(bv[sl] * scale)[None, :].astype(bft),
            "Wo": np.ascontiguousarray(Wo[sl, :]).astype(bft),
            "Wo4": np.ascontiguousarray(4.0 * Wo[sl, :]).astype(bft),
            "bo2": np.ascontiguousarray(
                (bo if h == 0 else np.zeros_like(bo)).reshape(4, 128).T
            ).astype(np.float32),
            "alpha": np.ascontiguousarray(alpha)[:, None],
            "beta4": np.ascontiguousarray(4.0 * beta)[:, None],
            "ident": ident, "identf": identf,
        })
    return in_maps


def assemble(results):
    out = np.zeros((BS, S, DM), np.float32)
    for r in results:
        out += np.swapaxes(np.asarray(r["outT"], np.float32), 1, 2)
    return out


def kernel(**inputs) -> np.ndarray:
    global _cached_nc
    from concourse.bass_utils import run_bass_kernel_spmd

    if _cached_nc is None:
        _cached_nc = _build()
    in_maps = make_in_maps(**inputs)
    res = run_bass_kernel_spmd(_cached_nc, in_maps, list(range(NCORES)))
    return assemble(res.results)



# revision 8
# speedup vs baseline: 1.1382x; 1.1382x over previous
"""Trainium2 Bass kernel for nn_MultiHeadCrossAttention (BS=4, S=512, DM=512, H=8).

Sharding: one attention head per NeuronCore (8 heads / 8 cores). Each core
receives the full (transposed) q/k/v plus its head's weight slices, computes
its head end-to-end including the rank-64 slice of the output projection, and
the host sums the 8 partial outputs.

v2 layout/engine plan:
  - weights packed into one bf16 blob + one f32 blob, DMA'd first
  - k/q projections emit [128,*] dup outputs (lhsT = [W|W]) so P2 can run
    kb-pairs concurrently in row groups {0-63} / {64-127}
  - fenmu = sum_kb exp(.) via bf16 identity-matmul PSUM accumulation
  - softmax/LN stats via grouped bn_stats (even/odd combine) + 1-NR rsqrt
  - w0 applied pre-transpose as a per-partition scalar (no rank-1 matmuls)
  - P6 single K=128 matmul per chunk: lhsT = [Wo;4Wo], rhs = [heads;qh]
  - output DRAM layout [B,128,4,S] (4KB/partition descriptors)
"""

import numpy as np

BS, S, DM, H, DK = 4, 512, 512, 8, 64
NCORES = 8
WCOLS = 512 + 512 + 256 + 512 + 64 + 128  # Wk,Wq,Wv,WoB,bvb,id = 1984
FCOLS = 8 + 128  # biases + idf


def build_program(nc, tile, mybir):
    f32 = mybir.dt.float32
    bf16 = mybir.dt.bfloat16
    i32 = mybir.dt.int32
    AF = mybir.ActivationFunctionType
    OP = mybir.AluOpType

    wb_d = nc.dram_tensor("wblob", [128, WCOLS], bf16, kind="ExternalInput")
    fb_d = nc.dram_tensor("fblob", [128, FCOLS], f32, kind="ExternalInput")
    kT_d = nc.dram_tensor("kT", [BS, 128, 4, S], bf16, kind="ExternalInput")
    qT_d = nc.dram_tensor("qT", [BS, 128, 4, S], bf16, kind="ExternalInput")
    vT_d = nc.dram_tensor("vT", [BS, 128, 4, S], bf16, kind="ExternalInput")
    outT_d = nc.dram_tensor("outT", [BS, 128, 4, S], bf16, kind="ExternalOutput")

    with tile.TileContext(nc) as tc:
        with (
            tc.tile_pool(name="persist", bufs=1) as pp,
            tc.tile_pool(name="inp", bufs=3) as inp,
            tc.tile_pool(name="ex", bufs=2) as exp_,
            tc.tile_pool(name="wk", bufs=2) as wkp,
            tc.tile_pool(name="st", bufs=1) as stp,
            tc.tile_pool(name="ob", bufs=2) as obp,
            tc.tile_pool(name="psum", bufs=1, space="PSUM") as psp,
        ):
            # ---- persistent SBUF ----
            wb = pp.tile([128, WCOLS], bf16, tag="wb")
            fb = pp.tile([128, FCOLS], f32, tag="fb")
            Wk_s = wb[:, 0:512].rearrange("p (m c) -> p m c", m=4)
            Wq_s = wb[:, 512:1024].rearrange("p (m c) -> p m c", m=4)
            Wv_s = wb[:, 1024:1280].rearrange("p (m c) -> p m c", m=4)
            WoB_s = wb[:, 1280:1792].rearrange("p (m c) -> p m c", m=4)
            bvb_s = wb[:, 1792:1856]
            id_s = wb[:, 1856:1984]
            bk2 = fb[:, 0:1]
            bq2 = fb[:, 1:2]
            bo4 = fb[:, 2:6]
            alx = fb[:, 6:7]
            b4x = fb[:, 7:8]
            idf_s = fb[:, 8:136]

            khT2 = pp.tile([128, BS, S], bf16, tag="khT2")
            qhT2 = pp.tile([128, BS, S], bf16, tag="qhT2")
            vh_all = pp.tile([128, 4, BS, DK], bf16, tag="vh")  # [j,jc,c,d]
            heads = pp.tile([128, BS, S], bf16, tag="heads")  # 0:64 ln, 64: qh
            Z_all = pp.tile([128, BS, 16], f32, tag="Z")   # (b, ic*4+c)
            Q_all = pp.tile([128, BS, 16], f32, tag="Q")
            w1_all = pp.tile([128, BS, 16], f32, tag="w1")
            w0_all = pp.tile([128, 16], f32, tag="w0")
            wup = pp.tile([1, 8], f32, tag="wup")

            def psum_pe():
                return psp.tile([128, 2, S], f32, tag="pe", bufs=2, name="ppe")

            def psum_fs(shape):
                return psp.tile(shape, f32, tag="fs", bufs=2, name="pfs")

            def psum_po(shape):
                return psp.tile(shape, f32, tag="po", bufs=2, name="ppo")

            # ---- prelude: ACT table preload + DMAs in use-order ----
            nc.vector.memset(wup[:], 1.0)
            nc.scalar.activation(wup[:], wup[:], AF.Exp)
            nc.sync.dma_start(wb[:], wb_d[:])
            nc.sync.dma_start(fb[:], fb_d[:])
            ktiles, qtiles, vtiles = [], [], []
            for b in range(BS):
                kt = inp.tile([128, 4, S], bf16, tag="kt", bufs=4,
                              name=f"kt{b}")
                nc.sync.dma_start(kt[:], kT_d[b])
                ktiles.append(kt)
            qt0 = inp.tile([128, 4, S], bf16, tag="qt", bufs=4, name="qt0")
            nc.sync.dma_start(qt0[:], qT_d[0])
            qtiles.append(qt0)
            for b in range(BS):
                vt = inp.tile([128, 4, S], bf16, tag="vt", bufs=4,
                              name=f"vt{b}")
                nc.sync.dma_start(vt[:], vT_d[b])
                vtiles.append(vt)
            for b in range(1, BS):
                qt = inp.tile([128, 4, S], bf16, tag="qt", bufs=4,
                              name=f"qt{b}")
                nc.sync.dma_start(qt[:], qT_d[b])
                qtiles.append(qt)

            # ---- phase emitters ----
            def emit_p1(W_s, bias_c, src, dst, b):
                ps = psum_po([128, S])
                for mc in range(4):
                    nc.tensor.matmul(ps[:], W_s[:, mc, :], src[:, mc, :],
                                     start=(mc == 0), stop=(mc == 3))
                nc.vector.tensor_scalar(dst[:, b, :], ps[:], bias_c, None,
                                        op0=OP.add)

            def emit_hq(b):
                nc.gpsimd.tensor_copy(heads[64:128, b, :], qhT2[64:128, b, :])

            def emit_vh(c, jc):
                pv = psum_po([128, DK])
                vt = vtiles[c]
                for mc in range(4):
                    nc.tensor.matmul(
                        pv[:], vt[:, mc, jc * 128:(jc + 1) * 128],
                        Wv_s[:, mc, :], start=(mc == 0), stop=(mc == 3))
                nc.vector.tensor_tensor(vh_all[:, jc, c, :], pv[:], bvb_s[:],
                                        op=OP.add)

            rtiles = {}

            def emit_p2(b, jc):
                jcs = slice(jc * 128, (jc + 1) * 128)
                exq = exp_.tile([128, 4, S], bf16, tag="ex", name="exq")
                for half in range(2):
                    ph = psum_pe()
                    kb0, kb1 = 2 * half, 2 * half + 1
                    nc.tensor.matmul(ph[:, 0, :], khT2[0:64, kb0, jcs],
                                     qhT2[0:64, b, :], start=True, stop=True)
                    nc.tensor.matmul(ph[:, 1, :], khT2[64:128, kb1, jcs],
                                     qhT2[64:128, b, :], start=True, stop=True)
                    nc.scalar.activation(
                        exq[:, 2 * half:2 * half + 2, :], ph[:], AF.Exp)
                fp = psum_fs([128, S])
                for kb in range(4):
                    nc.tensor.matmul(fp[:], id_s[:], exq[:, kb, :],
                                     start=(kb == 0), stop=(kb == 3))
                wrec = wkp.tile([128, S], f32, tag="wrec", name="wrec")
                nc.vector.reciprocal_approx_fast(wrec[:], fp[:])
                if jc == 0:
                    rtiles[b] = wkp.tile([128, 4, S], bf16, tag="rt",
                                         name=f"rt{b}")
                nc.gpsimd.tensor_tensor(rtiles[b][:, jc, :], exq[:, b, :],
                                        wrec[:], op=OP.mult)

            etiles = {}

            def emit_p3(b, p):
                rt = rtiles[b]
                if p == 0:
                    etiles[b] = wkp.tile([128, 4, BS, DK], bf16, tag="ea",
                                         bufs=3, name=f"ea{b}")
                ea = etiles[b]
                scp = psum_fs([128, 2, BS * DK])
                for u in range(2):
                    ic = 2 * p + u
                    ics = slice(ic * 128, (ic + 1) * 128)
                    for jc in range(4):
                        nc.tensor.matmul(
                            scp[:, u, :], rt[:, jc, ics],
                            vh_all[:, jc].rearrange("p c d -> p (c d)"),
                            start=(jc == 0), stop=(jc == 3))
                nc.scalar.activation(
                    ea[:, 2 * p:2 * p + 2].rearrange("p a c d -> p a (c d)"),
                    scp[:], AF.Exp)

            def emit_zq(b, eng):
                # e2 = e*e (gpsimd), then grouped reduces -> Z, Q (DVE-only)
                ea = etiles[b]
                e2 = wkp.tile([128, 16, DK], bf16, tag="e2", name="e2")
                eav = ea[:].rearrange("p a c d -> p (a c) d")
                eng.tensor_tensor(e2[:], eav, eav, op=OP.mult)
                nc.vector.tensor_reduce(Z_all[:, b, :], eav,
                                        axis=mybir.AxisListType.X, op=OP.add)
                nc.vector.tensor_reduce(Q_all[:, b, :], e2[:],
                                        axis=mybir.AxisListType.X, op=OP.add)

            SQ63 = float(np.sqrt(63.0))

            def emit_stats(hb):
                # b-pair hb: [128, 32] views; ln scale-invariance kills 1/Z:
                # w1 = sqrt(63)*rsqrt(Q - Z^2/64), w0 = -Z*w1/64
                bsl = slice(2 * hb, 2 * hb + 2)
                cnt = [128, 32]
                Zv = Z_all[:, bsl, :].rearrange("p b g -> p (b g)")
                Qv = Q_all[:, bsl, :].rearrange("p b g -> p (b g)")
                t_ = stp.tile(cnt, f32, tag=f"t{hb}", name="t_")
                nc.vector.tensor_tensor(t_[:], Zv, Zv, op=OP.mult)
                s_ = stp.tile(cnt, f32, tag=f"s{hb}", name="s_")
                nc.vector.scalar_tensor_tensor(s_[:], t_[:], -1.0 / DK, Qv,
                                               op0=OP.mult, op1=OP.add)
                # rsqrt seed + 1 NR iter
                r_ = stp.tile(cnt, f32, tag=f"r{hb}", name="r_")
                nc.vector.tensor_scalar(r_[:].bitcast(i32), s_[:].bitcast(i32),
                                        1, None, op0=OP.logical_shift_right)
                nc.vector.tensor_scalar(r_[:].bitcast(i32), r_[:].bitcast(i32),
                                        -1, 0x5F3759DF, op0=OP.mult, op1=OP.add)
                nt = stp.tile(cnt, f32, tag=f"n{hb}", name="nt")
                nc.vector.tensor_tensor(nt[:], s_[:], r_[:], op=OP.mult)
                nc.vector.tensor_tensor(nt[:], nt[:], r_[:], op=OP.mult)
                nc.vector.tensor_scalar(nt[:], nt[:], -0.5, 1.5,
                                        op0=OP.mult, op1=OP.add)
                nc.vector.tensor_tensor(r_[:], r_[:], nt[:], op=OP.mult)
                w1v = w1_all[:, bsl, :].rearrange("p b g -> p (b g)")
                nc.vector.tensor_scalar(w1v, r_[:], SQ63, None, op0=OP.mult)
                zg = stp.tile(cnt, f32, tag=f"zg{hb}", name="zg")
                nc.vector.tensor_tensor(zg[:], Zv, w1v, op=OP.mult)
                w0r = stp.tile([128, 8], f32, tag=f"w{hb}", name="w0r")
                nc.vector.tensor_reduce(
                    w0r[:], zg[:].rearrange("p (g c) -> p g c", c=4),
                    axis=mybir.AxisListType.X, op=OP.add)
                nc.vector.tensor_scalar(w0_all[:, 8 * hb:8 * hb + 8], w0r[:],
                                        -1.0 / DK, None, op0=OP.mult)

            def emit_p5(b):
                ea = etiles[b]
                bsc = obp.tile([128, 16, DK], bf16, tag="bsc", name="bsc")
                w1b = (w1_all[:, b, :].unsqueeze(-1)
                       .broadcast_to((128, 16, DK)))
                nc.gpsimd.tensor_tensor(
                    bsc[:], ea[:].rearrange("p a c d -> p (a c) d"), w1b,
                    op=OP.mult)
                bp = psum_fs([128, 4, DK])
                for ic in range(4):
                    for c in range(4):
                        nc.tensor.matmul(bp[:, ic, :], id_s[:],
                                         bsc[:, ic * 4 + c, :],
                                         start=(c == 0), stop=(c == 3))
                balls = obp.tile([128, 4, DK], f32, tag="balls", name="balls")
                for ic in range(4):
                    nc.vector.tensor_scalar(
                        balls[:, ic, :], bp[:, ic, :],
                        w0_all[:, b * 4 + ic:b * 4 + ic + 1], None, op0=OP.add)
                pt = psum_po([64, S])
                for ic in range(4):
                    nc.tensor.matmul(pt[0:64, ic * 128:(ic + 1) * 128],
                                     balls[:, ic, :], idf_s,
                                     is_transpose=True, start=True, stop=True)
                nc.vector.tensor_scalar(heads[0:64, b, :], pt[0:64, :],
                                        alx[0:64, :], b4x[0:64, :],
                                        op0=OP.mult, op1=OP.add)

            def emit_p6(b):
                osb = obp.tile([128, 4, S], bf16, tag="osb", name="osb")
                for nch in range(4):
                    pp6 = psum_po([128, S])
                    nc.tensor.matmul(pp6[:], WoB_s[:, nch, :], heads[:, b, :],
                                     start=True, stop=True)
                    if nch % 2 == 0:
                        nc.scalar.activation(osb[:, nch, :], pp6[:],
                                             AF.Identity,
                                             bias=bo4[:, nch:nch + 1])
                    else:
                        nc.vector.tensor_scalar(osb[:, nch, :], pp6[:],
                                                bo4[:, nch:nch + 1], None,
                                                op0=OP.add)
                nc.sync.dma_start(outT_d[b], osb[:])

            # ---- emission schedule ----
            for b in range(BS):
                emit_p1(Wk_s, bk2, ktiles[b], khT2, b)
            emit_p1(Wq_s, bq2, qtiles[0], qhT2, 0)
            emit_hq(0)
            emit_p2(0, 0)
            emit_vh(0, 0)
            emit_vh(0, 1)
            emit_p2(0, 1)
            emit_vh(0, 2)
            emit_vh(0, 3)
            emit_p1(Wq_s, bq2, qtiles[1], qhT2, 1)
            emit_hq(1)
            emit_p2(0, 2)
            emit_vh(1, 0)
            emit_vh(1, 1)
            emit_p2(0, 3)
            emit_vh(1, 2)
            emit_vh(1, 3)
            emit_p1(Wq_s, bq2, qtiles[2], qhT2, 2)
            emit_hq(2)
            emit_p2(1, 0)
            emit_vh(2, 0)
            emit_vh(2, 1)
            emit_p2(1, 1)
            emit_vh(2, 2)
            emit_vh(2, 3)
            emit_p1(Wq_s, bq2, qtiles[3], qhT2, 3)
            emit_hq(3)
            emit_p2(1, 2)
            emit_vh(3, 0)
            emit_vh(3, 1)
            emit_p2(1, 3)
            emit_vh(3, 2)
            emit_vh(3, 3)
            emit_p2(2, 0)
            emit_p3(0, 0)
            emit_p2(2, 1)
            emit_p3(0, 1)
            emit_zq(0, nc.gpsimd)
            emit_p2(2, 2)
            emit_p2(2, 3)
            emit_p2(3, 0)
            emit_p3(1, 0)
            emit_p2(3, 1)
            emit_p3(1, 1)
            emit_zq(1, nc.vector)
            emit_stats(0)
            emit_p2(3, 2)
            emit_p3(2, 0)
            emit_p2(3, 3)
            emit_p5(0)
            emit_p3(2, 1)
            emit_zq(2, nc.gpsimd)
            emit_p5(1)
            emit_p6(0)
            emit_p3(3, 0)
            emit_p3(3, 1)
            emit_zq(3, nc.vector)
            emit_stats(1)
            emit_p6(1)
            emit_p5(2)
            emit_p5(3)
            emit_p6(2)
            emit_p6(3)

    return nc


def _build():
    import concourse.bass as bass  # noqa
    import concourse.tile as tile
    from concourse import bacc, mybir

    nc = bacc.Bacc("TRN2", target_bir_lowering=False, debug=False,
                   num_devices=NCORES)
    build_program(nc, tile, mybir)
    nc.compile()
    return nc


_cached_nc = None


def make_in_maps(q, k, v, Wq, bq, Wk, bk, Wv, bv, Wo, bo, alpha, beta):
    import ml_dtypes
    bft = ml_dtypes.bfloat16

    def prelay(x):
        xT = np.swapaxes(np.asarray(x, np.float32), 1, 2)  # [B, DM, S]
        return np.ascontiguousarray(
            xT.reshape(BS, 4, 128, S).transpose(0, 2, 1, 3)).astype(bft)

    def wlay(W):  # [DM, DK] -> [128, 4, DK]
        return np.ascontiguousarray(
            np.asarray(W, np.float32).reshape(4, 128, DK).transpose(1, 0, 2))

    qT, kT, vT = prelay(q), prelay(k), prelay(v)
    Wq, Wk, Wv, Wo = (np.asarray(x, np.float32) for x in (Wq, Wk, Wv, Wo))
    bq, bk, bv, bo = (np.asarray(x, np.float32) for x in (bq, bk, bv, bo))
    alpha, beta = np.asarray(alpha, np.float32), np.asarray(beta, np.float32)
    scale = np.float32(1.0 / np.sqrt(np.float32(DK)))
    idbf = np.eye(128, dtype=np.float32)
    in_maps = []
    for h in range(NCORES):
        sl = slice(h * DK, (h + 1) * DK)
        WkD = wlay(Wk[:, sl])
        WkD = np.concatenate([WkD, WkD], axis=2).reshape(128, 512)
        WqD = wlay(Wq[:, sl])
        WqD = np.concatenate([WqD, WqD], axis=2).reshape(128, 512)
        WvS = wlay(Wv[:, sl] * scale).reshape(128, 256)
        WoStack = np.concatenate([Wo[sl, :], 4.0 * Wo[sl, :]], axis=0)
        WoB = WoStack.reshape(128, 4, 128).reshape(128, 512)
        bvb = np.tile((bv[sl] * scale)[None, :], (128, 1))
        wblob = np.ascontiguousarray(np.concatenate(
            [WkD, WqD, WvS, WoB, bvb, idbf], axis=1)).astype(bft)
        bo_h = bo if h == 0 else np.zeros_like(bo)
        alx = np.zeros(128, np.float32)
        alx[0:DK] = alpha
        b4x = np.zeros(128, np.float32)
        b4x[0:DK] = 4.0 * beta
        fblob = np.ascontiguousarray(np.concatenate(
            [np.tile(bk[sl], 2)[:, None], np.tile(bq[sl], 2)[:, None],
             bo_h.reshape(4, 128).T, alx[:, None], b4x[:, None], idbf],
            axis=1)).astype(np.float32)
        in_maps.append({
            "wblob": wblob, "fblob": fblob,
            "kT": kT, "qT": qT, "vT": vT,
        })
    return in_maps


def assemble(results):
    out = np.zeros((BS, S, DM), np.float32)
    for r in results:
        out += np.asarray(r["outT"], np.float32).transpose(0, 3, 2, 1).reshape(
            BS, S, DM)
    return out


def kernel(**inputs) -> np.ndarray:
    global _cached_nc
    from concourse.bass_utils import run_bass_kernel_spmd

    if _cached_nc is None:
        _cached_nc = _build()
    in_maps = make_in_maps(**inputs)
    res = run_bass_kernel_spmd(_cached_nc, in_maps, list(range(NCORES)))
    return assemble(res.results)


# revision 16
# speedup vs baseline: 1.1404x; 1.0019x over previous
"""Trainium2 Bass kernel for nn_MultiHeadCrossAttention (BS=4, S=512, DM=512, H=8).

Sharding: one attention head per NeuronCore (8 heads / 8 cores). Each core
receives the full (transposed) q/k/v plus its head's weight slices, computes
its head end-to-end including the rank-64 slice of the output projection, and
the host sums the 8 partial outputs.

v2 layout/engine plan:
  - weights packed into one bf16 blob + one f32 blob, DMA'd first
  - k/q projections emit [128,*] dup outputs (lhsT = [W|W]) so P2 can run
    kb-pairs concurrently in row groups {0-63} / {64-127}
  - fenmu = sum_kb exp(.) via bf16 identity-matmul PSUM accumulation
  - softmax/LN stats via grouped bn_stats (even/odd combine) + 1-NR rsqrt
  - w0 applied pre-transpose as a per-partition scalar (no rank-1 matmuls)
  - P6 single K=128 matmul per chunk: lhsT = [Wo;4Wo], rhs = [heads;qh]
  - output DRAM layout [B,128,4,S] (4KB/partition descriptors)
"""

import numpy as np

BS, S, DM, H, DK = 4, 512, 512, 8, 64
NCORES = 8
WCOLS = 512 + 512 + 256 + 512 + 64 + 128  # Wk,Wq,Wv,WoB,bvb,id = 1984
FCOLS = 8 + 128  # biases + idf


def build_program(nc, tile, mybir):
    f32 = mybir.dt.float32
    bf16 = mybir.dt.bfloat16
    i32 = mybir.dt.int32
    AF = mybir.ActivationFunctionType
    OP = mybir.AluOpType

    wb_d = nc.dram_tensor("wblob", [128, WCOLS], bf16, kind="ExternalInput")
    fb_d = nc.dram_tensor("fblob", [128, FCOLS], f32, kind="ExternalInput")
    kT_d = nc.dram_tensor("kT", [BS, 128, 4, S], bf16, kind="ExternalInput")
    qT_d = nc.dram_tensor("qT", [BS, 128, 4, S], bf16, kind="ExternalInput")
    vT_d = nc.dram_tensor("vT", [BS, 128, 4, S], bf16, kind="ExternalInput")
    outT_d = nc.dram_tensor("outT", [BS, 128, 4, S], bf16, kind="ExternalOutput")

    with tile.TileContext(nc) as tc:
        with (
            tc.tile_pool(name="persist", bufs=1) as pp,
            tc.tile_pool(name="inp", bufs=3) as inp,
            tc.tile_pool(name="ex", bufs=2) as exp_,
            tc.tile_pool(name="wk", bufs=2) as wkp,
            tc.tile_pool(name="st", bufs=1) as stp,
            tc.tile_pool(name="ob", bufs=2) as obp,
            tc.tile_pool(name="psum", bufs=1, space="PSUM") as psp,
        ):
            # ---- persistent SBUF ----
            wb = pp.tile([128, WCOLS], bf16, tag="wb")
            fb = pp.tile([128, FCOLS], f32, tag="fb")
            Wk_s = wb[:, 0:512].rearrange("p (m c) -> p m c", m=4)
            Wq_s = wb[:, 512:1024].rearrange("p (m c) -> p m c", m=4)
            Wv_s = wb[:, 1024:1280].rearrange("p (m c) -> p m c", m=4)
            WoB_s = wb[:, 1280:1792].rearrange("p (m c) -> p m c", m=4)
            bvb_s = wb[:, 1792:1856]
            id_s = wb[:, 1856:1984]
            bk2 = fb[:, 0:1]
            bq2 = fb[:, 1:2]
            bo4 = fb[:, 2:6]
            alx = fb[:, 6:7]
            b4x = fb[:, 7:8]
            idf_s = fb[:, 8:136]

            khT2 = pp.tile([128, BS, S], bf16, tag="khT2")
            qhT2 = pp.tile([128, BS, S], bf16, tag="qhT2")
            vh_all = pp.tile([128, 4, BS, DK], bf16, tag="vh")  # [j,jc,c,d]
            heads = pp.tile([128, BS, S], bf16, tag="heads")  # 0:64 ln, 64: qh
            Z_all = pp.tile([128, BS, 16], f32, tag="Z")   # (b, ic*4+c)
            Q_all = pp.tile([128, BS, 16], f32, tag="Q")
            w1_all = pp.tile([128, BS, 16], f32, tag="w1")
            w0_all = pp.tile([128, 16], bf16, tag="w0")
            wup = pp.tile([1, 8], f32, tag="wup")

            def psum_pe():
                return psp.tile([128, 2, S], f32, tag="pe", bufs=2, name="ppe")

            def psum_fs(shape):
                return psp.tile(shape, f32, tag="fs", bufs=2, name="pfs")

            def psum_po(shape):
                return psp.tile(shape, f32, tag="po", bufs=2, name="ppo")

            # ---- prelude: ACT table preload + DMAs in use-order ----
            nc.vector.memset(wup[:], 1.0)
            nc.scalar.activation(wup[:], wup[:], AF.Exp)
            nc.sync.dma_start(wb[:], wb_d[:])
            ktiles = [inp.tile([128, 4, S], bf16, tag="kt", bufs=4,
                               name=f"kt{b}") for b in range(BS)]
            qtiles = [inp.tile([128, 4, S], bf16, tag="qt", bufs=4,
                               name=f"qt{b}") for b in range(BS)]
            vtiles = [inp.tile([128, 4, S], bf16, tag="vt", bufs=4,
                               name=f"vt{b}") for b in range(BS)]
            # trigger order = sync-engine issue order: wblob first, then k
            # (P2 needs all of kh), q0, then v and the remaining q's.
            nc.sync.dma_start(ktiles[0][:], kT_d[0])
            nc.sync.dma_start(ktiles[1][:], kT_d[1])
            nc.sync.dma_start(qtiles[0][:], qT_d[0])
            nc.sync.dma_start(ktiles[2][:], kT_d[2])
            nc.sync.dma_start(ktiles[3][:], kT_d[3])
            nc.sync.dma_start(qtiles[1][:], qT_d[1])
            nc.sync.dma_start(fb[:], fb_d[:])
            nc.sync.dma_start(vtiles[0][:], vT_d[0])
            nc.sync.dma_start(vtiles[1][:], vT_d[1])
            nc.sync.dma_start(qtiles[2][:], qT_d[2])
            nc.sync.dma_start(vtiles[2][:], vT_d[2])
            nc.sync.dma_start(vtiles[3][:], vT_d[3])
            nc.sync.dma_start(qtiles[3][:], qT_d[3])

            # ---- phase emitters ----
            def emit_p1(W_s, bias_c, src, dst, b):
                ps = psum_po([128, S])
                for mc in range(4):
                    nc.tensor.matmul(ps[:], W_s[:, mc, :], src[:, mc, :],
                                     start=(mc == 0), stop=(mc == 3))
                nc.vector.tensor_scalar(dst[:, b, :], ps[:], bias_c, None,
                                        op0=OP.add)

            def emit_hq(b):
                nc.vector.tensor_copy(heads[64:128, b, :], qhT2[64:128, b, :])

            def emit_vh(c):
                pv = psum_po([128, 4, DK])
                vt = vtiles[c]
                for jc in range(4):
                    for mc in range(4):
                        nc.tensor.matmul(
                            pv[:, jc, :], vt[:, mc, jc * 128:(jc + 1) * 128],
                            Wv_s[:, mc, :], start=(mc == 0), stop=(mc == 3))
                nc.vector.tensor_tensor(
                    vh_all[:, :, c, :], pv[:],
                    bvb_s[:].unsqueeze(1).broadcast_to((128, 4, DK)),
                    op=OP.add)

            rtiles = {}

            def emit_p2(b, jc):
                jcs = slice(jc * 128, (jc + 1) * 128)
                exq = exp_.tile([128, 4, S], bf16, tag="ex", name="exq")
                for half in range(2):
                    ph = psum_pe()
                    kb0, kb1 = 2 * half, 2 * half + 1
                    nc.tensor.matmul(ph[:, 0, :], khT2[0:64, kb0, jcs],
                                     qhT2[0:64, b, :], start=True, stop=True)
                    nc.tensor.matmul(ph[:, 1, :], khT2[64:128, kb1, jcs],
                                     qhT2[64:128, b, :], start=True, stop=True)
                    nc.scalar.activation(
                        exq[:, 2 * half:2 * half + 2, :], ph[:], AF.Exp)
                fp = psum_fs([128, S])
                for kb in range(4):
                    nc.tensor.matmul(fp[:], id_s[:], exq[:, kb, :],
                                     start=(kb == 0), stop=(kb == 3))
                wrec = wkp.tile([128, S], f32, tag="wrec", name="wrec")
                nc.vector.reciprocal_approx_fast(wrec[:], fp[:])
                if jc == 0:
                    rtiles[b] = wkp.tile([128, 4, S], bf16, tag="rt",
                                         name=f"rt{b}")
                nc.gpsimd.tensor_tensor(rtiles[b][:, jc, :], exq[:, b, :],
                                        wrec[:], op=OP.mult)

            etiles = {}

            def emit_p3(b, p):
                rt = rtiles[b]
                if p == 0:
                    etiles[b] = wkp.tile([128, 4, BS, DK], bf16, tag="ea",
                                         bufs=3, name=f"ea{b}")
                ea = etiles[b]
                scp = psum_fs([128, 2, BS * DK])
                for u in range(2):
                    ic = 2 * p + u
                    ics = slice(ic * 128, (ic + 1) * 128)
                    for jc in range(4):
                        nc.tensor.matmul(
                            scp[:, u, :], rt[:, jc, ics],
                            vh_all[:, jc].rearrange("p c d -> p (c d)"),
                            start=(jc == 0), stop=(jc == 3))
                nc.scalar.activation(
                    ea[:, 2 * p:2 * p + 2].rearrange("p a c d -> p a (c d)"),
                    scp[:], AF.Exp)

            def emit_zq(b, eng):
                # e2 = e*e (gpsimd), then grouped reduces -> Z, Q (DVE-only)
                ea = etiles[b]
                e2 = wkp.tile([128, 16, DK], bf16, tag="e2", name="e2")
                eav = ea[:].rearrange("p a c d -> p (a c) d")
                eng.tensor_tensor(e2[:], eav, eav, op=OP.mult)
                nc.vector.tensor_reduce(Z_all[:, b, :], eav,
                                        axis=mybir.AxisListType.X, op=OP.add)
                nc.vector.tensor_reduce(Q_all[:, b, :], e2[:],
                                        axis=mybir.AxisListType.X, op=OP.add)

            SQ63 = float(np.sqrt(63.0))

            def emit_stats(b0, nb):
                # batches [b0, b0+nb): ln scale-invariance kills 1/Z:
                # w1 = sqrt(63)*rsqrt(Q - Z^2/64), w0 = -Z*w1/64
                bsl = slice(b0, b0 + nb)
                cnt = [128, 16 * nb]
                Zv = Z_all[:, bsl, :].rearrange("p b g -> p (b g)")
                Qv = Q_all[:, bsl, :].rearrange("p b g -> p (b g)")
                t_ = stp.tile(cnt, f32, tag=f"t{b0}", name="t_")
                nc.vector.tensor_tensor(t_[:], Zv, Zv, op=OP.mult)
                s_ = stp.tile(cnt, f32, tag=f"s{b0}", name="s_")
                nc.vector.scalar_tensor_tensor(s_[:], t_[:], -1.0 / DK, Qv,
                                               op0=OP.mult, op1=OP.add)
                # rsqrt seed + 1 NR iter (w1 fused into the final mult)
                r_ = stp.tile(cnt, f32, tag=f"r{b0}", name="r_")
                nc.vector.tensor_scalar(r_[:].bitcast(i32), s_[:].bitcast(i32),
                                        1, None, op0=OP.logical_shift_right)
                nc.vector.tensor_scalar(r_[:].bitcast(i32), r_[:].bitcast(i32),
                                        -1, 0x5F3759DF, op0=OP.mult, op1=OP.add)
                nt = stp.tile(cnt, f32, tag=f"n{b0}", name="nt")
                nc.vector.tensor_tensor(nt[:], s_[:], r_[:], op=OP.mult)
                nc.vector.tensor_tensor(nt[:], nt[:], r_[:], op=OP.mult)
                nc.vector.tensor_scalar(nt[:], nt[:], -0.5, 1.5,
                                        op0=OP.mult, op1=OP.add)
                w1v = w1_all[:, bsl, :].rearrange("p b g -> p (b g)")
                nc.vector.scalar_tensor_tensor(w1v, r_[:], SQ63, nt[:],
                                               op0=OP.mult, op1=OP.mult)
                zg = stp.tile(cnt, f32, tag=f"zg{b0}", name="zg")
                nc.vector.scalar_tensor_tensor(zg[:], Zv, -1.0 / DK, w1v,
                                               op0=OP.mult, op1=OP.mult)
                w0r = stp.tile([128, 4 * nb], f32, tag=f"w{b0}", name="w0r")
                nc.vector.tensor_reduce(
                    w0r[:], zg[:].rearrange("p (g c) -> p g c", c=4),
                    axis=mybir.AxisListType.X, op=OP.add)
                nc.vector.tensor_copy(w0_all[:, 4 * b0:4 * (b0 + nb)], w0r[:])

            def emit_p5(b):
                ea = etiles[b]
                bsc = obp.tile([128, 16, DK], bf16, tag="bsc", name="bsc")
                w1b = (w1_all[:, b, :].unsqueeze(-1)
                       .broadcast_to((128, 16, DK)))
                nc.gpsimd.tensor_tensor(
                    bsc[:], ea[:].rearrange("p a c d -> p (a c) d"), w1b,
                    op=OP.mult)
                bp = psum_fs([128, 4, DK])
                for ic in range(4):
                    for c in range(4):
                        nc.tensor.matmul(bp[:, ic, :], id_s[:],
                                         bsc[:, ic * 4 + c, :],
                                         start=(c == 0), stop=False)
                    # += w0 broadcast along d via a 0-stride rhs
                    nc.tensor.matmul(
                        bp[:, ic, :], id_s[:],
                        w0_all[:, b * 4 + ic:b * 4 + ic + 1]
                        .broadcast_to((128, DK)),
                        start=False, stop=True)
                balls = obp.tile([128, 4, DK], f32, tag="balls", name="balls")
                nc.vector.tensor_copy(balls[:], bp[:])
                pt = psum_po([64, S])
                for ic in range(4):
                    nc.tensor.matmul(pt[0:64, ic * 128:(ic + 1) * 128],
                                     balls[:, ic, :], idf_s,
                                     is_transpose=True, start=True, stop=True)
                nc.vector.tensor_scalar(heads[0:64, b, :], pt[0:64, :],
                                        alx[0:64, :], b4x[0:64, :],
                                        op0=OP.mult, op1=OP.add)

            def emit_p6(b):
                osb = obp.tile([128, 4, S], bf16, tag="osb", name="osb")
                for nch in range(4):
                    pp6 = psum_po([128, S])
                    nc.tensor.matmul(pp6[:], WoB_s[:, nch, :], heads[:, b, :],
                                     start=True, stop=True)
                    if nch % 2 == 0:
                        nc.scalar.activation(osb[:, nch, :], pp6[:],
                                             AF.Identity,
                                             bias=bo4[:, nch:nch + 1])
                    else:
                        nc.vector.tensor_scalar(osb[:, nch, :], pp6[:],
                                                bo4[:, nch:nch + 1], None,
                                                op0=OP.add)
                nc.sync.dma_start(outT_d[b], osb[:])

            # ---- emission schedule ----
            emit_p1(Wk_s, bk2, ktiles[0], khT2, 0)
            emit_p1(Wk_s, bk2, ktiles[1], khT2, 1)
            emit_p1(Wq_s, bq2, qtiles[0], qhT2, 0)
            emit_p1(Wk_s, bk2, ktiles[2], khT2, 2)
            emit_p1(Wk_s, bk2, ktiles[3], khT2, 3)
            emit_hq(0)
            emit_p2(0, 0)
            emit_p1(Wq_s, bq2, qtiles[1], qhT2, 1)
            emit_hq(1)
            emit_p2(0, 1)
            emit_vh(0)
            emit_p2(0, 2)
            emit_vh(1)
            emit_p2(0, 3)
            emit_p1(Wq_s, bq2, qtiles[2], qhT2, 2)
            emit_hq(2)
            emit_p2(1, 0)
            emit_vh(2)
            emit_p2(1, 1)
            emit_vh(3)
            emit_p2(1, 2)
            emit_p1(Wq_s, bq2, qtiles[3], qhT2, 3)
            emit_hq(3)
            emit_p2(1, 3)
            emit_p3(0, 0)
            emit_p2(2, 0)
            emit_p3(0, 1)
            emit_p2(2, 1)
            emit_zq(0, nc.vector)
            emit_p2(2, 2)
            emit_p3(1, 0)
            emit_p2(2, 3)
            emit_p3(1, 1)
            emit_p2(3, 0)
            emit_zq(1, nc.vector)
            emit_stats(0, 2)
            emit_p2(3, 1)
            emit_p3(2, 0)
            emit_p2(3, 2)
            emit_p5(0)
            emit_p3(2, 1)
            emit_p2(3, 3)
            emit_zq(2, nc.vector)
            emit_stats(2, 1)
            emit_p6(0)
            emit_p5(1)
            emit_p3(3, 0)
            emit_p5(2)
            emit_p3(3, 1)
            emit_p6(1)
            emit_zq(3, nc.vector)
            emit_stats(3, 1)
            emit_p6(2)
            emit_p5(3)
            emit_p6(3)

    return nc


def _build():
    import concourse.bass as bass  # noqa
    import concourse.tile as tile
    from concourse import bacc, mybir

    nc = bacc.Bacc("TRN2", target_bir_lowering=False, debug=False,
                   num_devices=NCORES)
    build_program(nc, tile, mybir)
    nc.compile()
    return nc


_cached_nc = None


def make_in_maps(q, k, v, Wq, bq, Wk, bk, Wv, bv, Wo, bo, alpha, beta):
    import ml_dtypes
    bft = ml_dtypes.bfloat16

    def prelay(x):
        xT = np.swapaxes(np.asarray(x, np.float32), 1, 2)  # [B, DM, S]
        return np.ascontiguousarray(
            xT.reshape(BS, 4, 128, S).transpose(0, 2, 1, 3)).astype(bft)

    def wlay(W):  # [DM, DK] -> [128, 4, DK]
        return np.ascontiguousarray(
            np.asarray(W, np.float32).reshape(4, 128, DK).transpose(1, 0, 2))

    qT, kT, vT = prelay(q), prelay(k), prelay(v)
    Wq, Wk, Wv, Wo = (np.asarray(x, np.float32) for x in (Wq, Wk, Wv, Wo))
    bq, bk, bv, bo = (np.asarray(x, np.float32) for x in (bq, bk, bv, bo))
    alpha, beta = np.asarray(alpha, np.float32), np.asarray(beta, np.float32)
    scale = np.float32(1.0 / np.sqrt(np.float32(DK)))
    idbf = np.eye(128, dtype=np.float32)
    in_maps = []
    for h in range(NCORES):
        sl = slice(h * DK, (h + 1) * DK)
        WkD = wlay(Wk[:, sl])
        WkD = np.concatenate([WkD, WkD], axis=2).reshape(128, 512)
        WqD = wlay(Wq[:, sl])
        WqD = np.concatenate([WqD, WqD], axis=2).reshape(128, 512)
        WvS = wlay(Wv[:, sl] * scale).reshape(128, 256)
        WoStack = np.concatenate([Wo[sl, :], 4.0 * Wo[sl, :]], axis=0)
        WoB = WoStack.reshape(128, 4, 128).reshape(128, 512)
        bvb = np.tile((bv[sl] * scale)[None, :], (128, 1))
        wblob = np.ascontiguousarray(np.concatenate(
            [WkD, WqD, WvS, WoB, bvb, idbf], axis=1)).astype(bft)
        bo_h = bo if h == 0 else np.zeros_like(bo)
        alx = np.zeros(128, np.float32)
        alx[0:DK] = alpha
        b4x = np.zeros(128, np.float32)
        b4x[0:DK] = 4.0 * beta
        fblob = np.ascontiguousarray(np.concatenate(
            [np.tile(bk[sl], 2)[:, None], np.tile(bq[sl], 2)[:, None],
             bo_h.reshape(4, 128).T, alx[:, None], b4x[:, None], idbf],
            axis=1)).astype(np.float32)
        in_maps.append({
            "wblob": wblob, "fblob": fblob,
            "kT": kT, "qT": qT, "vT": vT,
        })
    return in_maps


def assemble(results):
    out = np.zeros((BS, S, DM), np.float32)
    for r in results:
        out += np.asarray(r["outT"], np.float32).transpose(0, 3, 2, 1).reshape(
            BS, S, DM)
    return out


def kernel(**inputs) -> np.ndarray:
    global _cached_nc
    from concourse.bass_utils import run_bass_kernel_spmd

    if _cached_nc is None:
        _cached_nc = _build()
    in_maps = make_in_maps(**inputs)
    res = run_bass_kernel_spmd(_cached_nc, in_maps, list(range(NCORES)))
    return assemble(res.results)
